# revision 5
# baseline (speedup 1.0000x reference)
"""Trainium2 Bass kernel for BiLinearInteractionLayer.

Computes, for every field pair p=(i,j), i<j, of F=32 fields:
    y[b, p, :] = (x[b, i, :] @ W[p].T) * x[b, j, :]
x: [4096, 32, 64] f32, W: [496, 64, 64] f32 -> y: [4096, 496, 64] f32.

Sharding: data-parallel over the batch dim across 8 NeuronCores (512
rows each); the weight stack is replicated.

Per-core algorithm (batch tile of 128 rows at a time):
  - Host pre-transposes layouts (free): the contraction dim d lands on
    SBUF partitions with clean contiguous DMAs, no on-device transposes.
  - For each first-field i, the pairs (i, i+1..31) are contiguous both in
    the pair axis and in the transposed weight columns: one stationary
    xT_i [64d, 128b] serves matmuls streaming W^T columns (N<=512 per
    PSUM bank) into a 4-bank PSUM group [128, (31-i)*64].
  - Even fields live on SBUF partitions 0-63 (PE row group 0), odd
    fields on 64-127 (row group 2): the two K=64 fp32 matmul streams
    execute on disjoint halves of the PE array and overlap, which
    matters because fp32 matmul costs 4 cycles/column.
  - The xj factors of a run are x[b, (i+1)*64 : 32*64] -- one contiguous
    slice -- so a single DVE tensor_mul fuses the PSUM read, the
    elementwise multiply and the SBUF write.
  - One output DMA per (tile, i): up to ~1 MB, 128 rows x (31-i)*256B.
"""

import itertools

import numpy as np

import concourse.bass as bass
import concourse.mybir as mybir
import concourse.tile as _tile
from concourse.bass_utils import run_bass_kernel_spmd
from concourse.tile import TileContext
from concourse.tile_scheduler import N_PROCS
from concourse.vector_clock import ScopedClock, VectorClock

# --------------------------------------------------------------------------
# Tail-drain patch: the staged walrus rejects >1 sync-wait command on a
# TPB_CTRL (Drain) instruction, but the stock Tile tail-drain attaches one
# wait per outstanding sem lane to a single Drain. Replace it with a ladder
# of single-wait SP nops (one per proc lane) followed by a wait-less drain.
# --------------------------------------------------------------------------


def _split_drain_and_barrier(self, tick_clock, wait_clock):
    nc = self.nc
    g = tick_clock.global_clock
    for p in range(N_PROCS):
        tick = g.peek_next(p) - 1
        if tick <= 0:
            continue
        pc = VectorClock()
        pc.require_at_least(p, tick)
        w = nc.sync.nop(nofuse=True)
        wait_clock.add_sem_waits(w.ins, ScopedClock({None: pc}))
    nc.sync.drain()
    nc.all_engine_barrier()
    assert self.sems is not None
    popped = nc._tile_sem_poison_stack.pop()
    assert popped is self._sem_poison
    nc.clear_and_free_semaphores(list(self.sems.allocated().values()))
    nc.all_engine_barrier()


_tile.TileContext._drain_and_barrier = _split_drain_and_barrier

_wsplit_counter = [0]


def _legalize_single_wait(nc):
    """Hoist extra sem waits onto preceding same-engine NoOps.

    This walrus build encodes at most ONE sync-wait command per TPB
    instruction; Tile's sem-assignment pass freely attaches several.
    Splitting extras onto immediately-preceding NoOps on the same engine
    preserves program order (engines issue in order), hence semantics."""
    import bass_rust

    for fn in nc.m.functions:
        for blk in fn.blocks:
            insts = list(blk.instructions)
            if not any(
                ins.sync_info is not None and len(ins.sync_info.on_wait) > 1
                for ins in insts
            ):
                continue
            out = []
            for ins in insts:
                si = ins.sync_info
                waits = list(si.on_wait) if si is not None else []
                if len(waits) > 1:
                    for w in waits[:-1]:
                        _wsplit_counter[0] += 1
                        nop = mybir.InstNoOp(
                            name=f"I-wsplit-{_wsplit_counter[0]}", ins=[], outs=[]
                        )
                        nop.engine = ins.engine
                        nop.sync_info = bass_rust.SyncInfo(
                            on_wait=[w], on_update=[]
                        )
                        out.append(nop)
                    si.on_wait = [waits[-1]]
                out.append(ins)
            blk.instructions = out


# --------------------------------------------------------------------------
# Problem constants (hardcoded per contract: kernel.py is self-contained).
# --------------------------------------------------------------------------
B, F, D = 4096, 32, 64
NCORES = 8
BL = B // NCORES          # 512 batch rows per core
PT = 128                  # batch tile = SBUF partition count
TILES = BL // PT          # 4 tiles per core
NPAIR = F * (F - 1) // 2  # 496
# pair index of (i, i+1) within itertools.combinations(range(F), 2) order
IDX0 = [0] * F
for _i in range(1, F):
    IDX0[_i] = IDX0[_i - 1] + (F - _i)
# per-parity column offset of field i's run inside its wt half
POFF = [0] * F
for _i in range(2, F):
    POFF[_i] = POFF[_i - 2] + (F - 1 - (_i - 2)) * D
WT_COLS = max(POFF[30] + 1 * D, POFF[31])  # even half is the larger: 16384
WT_COLS = max(WT_COLS, 16384)

F32 = mybir.dt.float32

_nc_cache = {}


def _build_bass(mm_dt=F32):
    nc = bass.Bass(trn_type="TRN2")
    x_d = nc.dram_tensor("x", [BL, F * D], F32, kind="ExternalInput")
    xt_d = nc.dram_tensor("xt", [PT, TILES * (F // 2) * PT], mm_dt,
                          kind="ExternalInput")
    wt_d = nc.dram_tensor("wt", [PT, WT_COLS], mm_dt, kind="ExternalInput")
    y_d = nc.dram_tensor("y", [BL, NPAIR * D], F32, kind="ExternalOutput")

    CB = (F // 2) * PT  # 2048 xt cols per batch tile

    with TileContext(nc) as tc:
        with (
            tc.tile_pool(name="wtp", bufs=1) as wtp,
            tc.tile_pool(name="iop", bufs=2) as iop,
            tc.tile_pool(name="outp", bufs=3) as outp,
            tc.tile_pool(name="pp", bufs=1, space="PSUM") as pp,
        ):
            wt_s = wtp.tile([PT, WT_COLS], mm_dt)
            # chunked weight load: matmuls for early fields only depend on
            # their own column range (Tile subtile deps), so compute starts
            # after ~1/8 of the weights have landed instead of all 8 MB
            WCH = 2048
            for w0 in range(0, WT_COLS, WCH):
                nc.sync.dma_start(
                    out=wt_s[:, w0 : w0 + WCH], in_=wt_d[:, w0 : w0 + WCH]
                )
            for t in range(TILES):
                x_s = iop.tile([PT, F * D], F32, tag="x")
                nc.sync.dma_start(out=x_s, in_=x_d[t * PT : (t + 1) * PT, :])
                xt_s = iop.tile([PT, CB], mm_dt, tag="xt")
                nc.sync.dma_start(out=xt_s, in_=xt_d[:, t * CB : (t + 1) * CB])
                for m in range(F // 2):
                    work = []  # (par, ps, out_s, ncol, c0) per live parity
                    for par in (0, 1):
                        i = 2 * m + par
                        if i > F - 2:
                            continue
                        ncol = (F - 1 - i) * D
                        ps = pp.tile(
                            [PT, 1984], F32, tag=f"ps{par}", name=f"ps_{t}_{i}"
                        )
                        out_s = outp.tile(
                            [PT, 1984], F32, tag=f"o{par}", name=f"o_{t}_{i}"
                        )
                        work.append((par, i, ps, out_s, ncol))
                    # interleave the two parities' matmul chunks so the two
                    # PE row groups stream concurrently
                    chunk_lists = []
                    for par, i, ps, out_s, ncol in work:
                        lhsT = xt_s[par * D : (par + 1) * D,
                                    m * PT : (m + 1) * PT]
                        off = POFF[i]
                        chunks = []
                        for k0 in range(0, ncol, 512):
                            kn = min(512, ncol - k0)
                            chunks.append((ps, lhsT, par, off, k0, kn))
                        chunk_lists.append(chunks)
                    for group in itertools.zip_longest(*chunk_lists):
                        for ch in group:
                            if ch is None:
                                continue
                            ps, lhsT, par, off, k0, kn = ch
                            nc.tensor.matmul(
                                ps[:, k0 : k0 + kn],
                                lhsT,
                                wt_s[par * D : (par + 1) * D,
                                     off + k0 : off + k0 + kn],
                                start=True,
                                stop=True,
                            )
                    for par, i, ps, out_s, ncol in work:
                        c0 = IDX0[i] * D
                        nc.vector.tensor_mul(
                            out=out_s[:, :ncol],
                            in0=ps[:, :ncol],
                            in1=x_s[:, (i + 1) * D : F * D],
                        )
                        nc.sync.dma_start(
                            out=y_d[t * PT : (t + 1) * PT, c0 : c0 + ncol],
                            in_=out_s[:, :ncol],
                        )
    _legalize_single_wait(nc)
    return nc


def _get_nc(mm_dt=F32):
    key = str(mm_dt)
    if key not in _nc_cache:
        _nc_cache[key] = _build_bass(mm_dt)
    return _nc_cache[key]


def _prep_inputs(x, W):
    x = np.ascontiguousarray(np.asarray(x, dtype=np.float32))
    W = np.ascontiguousarray(np.asarray(W, dtype=np.float32))
    # wt2[par*64+d, POFF[i] + (j-i-1)*64 + o] = W[(i,j), o, d]
    wt2 = np.zeros((PT, WT_COLS), dtype=np.float32)
    for i in range(F - 1):
        par = i % 2
        npair = F - 1 - i
        blk = W[IDX0[i] : IDX0[i] + npair]           # [npair, D, D]
        blk = blk.transpose(2, 0, 1).reshape(D, npair * D)
        wt2[par * D : (par + 1) * D, POFF[i] : POFF[i] + npair * D] = blk
    in_maps = []
    for c in range(NCORES):
        xl = x[c * BL : (c + 1) * BL]                      # [512, 32, 64]
        x_in = np.ascontiguousarray(xl.reshape(BL, F * D))
        # xt2[par*64+d, t*2048 + m*128 + b] = xl[t*128+b, 2m+par, d]
        xt2 = np.ascontiguousarray(
            xl.reshape(TILES, PT, F // 2, 2, D).transpose(3, 4, 0, 2, 1)
        ).reshape(PT, TILES * (F // 2) * PT)
        in_maps.append({"x": x_in, "xt": xt2, "wt": wt2})
    return in_maps


def _run(x, W, trace=False, mm_dt=F32):
    nc = _get_nc(mm_dt)
    in_maps = _prep_inputs(x, W)
    res = run_bass_kernel_spmd(nc, in_maps, core_ids=list(range(NCORES)), trace=trace)
    y = np.concatenate(
        [res.results[c]["y"].reshape(BL, NPAIR, D) for c in range(NCORES)], axis=0
    )
    return y, res


def kernel(x, W):
    y, _ = _run(x, W)
    return y
